# revision 4
# baseline (speedup 1.0000x reference)
"""MinGRU kernel for Trainium2 (8 NeuronCores, Bass/Tile) — v4.

Reference computation (B=4, L=8192, D=512, fp32):
    gates = sigmoid(x @ Wg.T + bg)
    cands = tanh(x @ Wc.T + bc)
    h_t   = (1 - g_t) * h_{t-1} + g_t * c_t   (scan along L, h_0 = 0)

Sharding: core c -> (batch b = c//2, channel half = c%2). Each core computes
its batch's full L range for 256 of the 512 output channels; the scan along L
is per (b, channel) so no cross-core communication is needed.

v4 over v3 (82.5us):
  * Matmuls in fp8e4m3 DoubleRow perf mode: x and W split hi+lo (shared
    scale, x*2^5 / W*2^12, undone by the activation scale 2^-17) and the
    three products xh*Wh + xh*Wl + xl*Wh accumulate in PSUM (the lo*lo
    term is dropped; simulated end-to-end rel err 4-5e-3 vs the 2e-2
    gate). 6 DoubleRow matmuls replace 4 fp16 matmuls per 512-token
    chunk: 1536 PE cycles vs 2048, and if DoubleRow runs at the cost
    model's 0.5 cycles/row the PE drops from 6.67 to 5.0 ns/token --
    below the DVE's ~7 ns/token, so the DVE never waits on activations
    after the ramp.
  * bn = (a-1)*c as one scalar_tensor_tensor instead of ts_sub+tt_mul:
    same DVE payload, one less instruction + semaphore set per unit.
  * Warm-up burst cut to 12 matmuls (v3's 33 overshot the first x
    arrival by ~3.5us in traces); segment ramp starts at 256 tokens so
    the first scan starts as soon as the first activation can finish.
  * Weights ship as one packed tensor (wg|wc) before x; bias first.
"""

import os
import sys

sys.path.insert(0, "/opt/trn_rl_repo")

import ml_dtypes
import numpy as np

import concourse.bacc as bacc
import concourse.bass as bass
import concourse.mybir as mybir
from concourse.bass_utils import run_bass_kernel_spmd
from concourse.tile import TileContext

B, L, D = 4, 8192, 512
NCORES = 8
EH = D // 2          # output channels per core
NET = EH // 128      # e-tiles per core (2)
NDC = D // 128       # contraction chunks (4)
NSUB = 512           # one fp32 PSUM bank of tokens (matmul N limit)
PSEG = 1024          # tokens per PSUM tile / ACT instruction
SEGS = [256, 384, 768, 1280, 1792, 2048, 1408, 256]
assert sum(SEGS) == L
MAXSEG = max(SEGS)

USE_FP8 = False
GPSIMD_STT = False  # run bn = (a-1)*c on GpSimd instead of DVE
SX = 2.0 ** 5        # x quant scale (|x*32| < 448)
SW = 2.0 ** 12       # W quant scale (|W*4096| < 448)
ACT_SCALE = 1.0 / (SX * SW) if USE_FP8 else 1.0

FP32 = mybir.dt.float32
F16 = mybir.dt.float16
F8 = mybir.dt.float8e4
_last_results = None

N_WARMUP_MM = 12


def build_nc() -> bass.Bass:
    # Bacc (not plain Bass): its compile() runs move_matmul_waits_to_ldweights
    # and generate_event_semaphores, which split multi-sem waits to satisfy the
    # TRN2 per-instruction wait-slot limits walrus enforces.
    nc = bacc.Bacc()

    if USE_FP8:
        # hi/lo pairs along dim1; dc pairs along dim2 feed DoubleRow matmuls
        xr = nc.dram_tensor("xr", [128, 2, NDC, L], F8, kind="ExternalInput")
        w8 = nc.dram_tensor("w8", [128, 2, NDC, 2 * EH], F8, kind="ExternalInput")
    else:
        xr = nc.dram_tensor("xr", [128, NDC, L], F16, kind="ExternalInput")
        w8 = nc.dram_tensor("w8", [128, NDC, 2 * EH], F16, kind="ExternalInput")
    # bias packed [128, 4]: cols 0..1 = -bg per e-tile, 2..3 = bc per e-tile
    bias = nc.dram_tensor("bias", [128, 2 * NET], FP32, kind="ExternalInput")
    h = nc.dram_tensor("h", [NET, 128, L], F16, kind="ExternalOutput")
    h_pel = h.rearrange("e p l -> p e l")

    op = mybir.AluOpType
    act = mybir.ActivationFunctionType
    DR = mybir.MatmulPerfMode.DoubleRow

    with TileContext(nc) as tc:
        with (
            tc.tile_pool(name="consts", bufs=1) as consts,
            tc.tile_pool(name="xpool", bufs=3) as xpool,
            tc.tile_pool(name="work", bufs=4) as work,
            tc.tile_pool(name="bnpool", bufs=2) as bnpool,
            tc.tile_pool(name="hpool", bufs=3) as hpool,
            tc.tile_pool(name="psum", bufs=2, space="PSUM") as psum,
        ):
            # PE warm-up: zero a dummy tile, then issue back-to-back matmuls
            # on it while the first weight/x DMAs are still in flight, so
            # PE_HAM releases the cold clock gate before the real stream.
            dummy = consts.tile([128, 128], F16)
            nc.vector.memset(dummy, 0.0)
            warm_ps = psum.tile([128, PSEG], FP32, tag="pg", name="warm")
            for _ in range(N_WARMUP_MM):
                nc.tensor.matmul(
                    warm_ps[:, 0:128], dummy, dummy, start=True, stop=True
                )

            # Everything rides the sync HWDGE ring. Queue order: bias ->
            # packed weights -> x segments in order -> h stores (emitted
            # after, so they can never head-of-line block an x load).
            bias_sb = consts.tile([128, 2 * NET], FP32)
            nc.sync.dma_start(bias_sb, bias[:])
            if USE_FP8:
                w_sb = consts.tile([128, 2, NDC, 2 * EH], F8)
            else:
                w_sb = consts.tile([128, NDC, 2 * EH], F16)
            nc.sync.dma_start(w_sb, w8[:])
            if USE_FP8:
                x_tiles = [
                    xpool.tile([128, 2, NDC, MAXSEG], F8, tag="x", name=f"x_{t}")[
                        :, :, :, :lt
                    ]
                    for t, lt in enumerate(SEGS)
                ]
            else:
                x_tiles = [
                    xpool.tile([128, NDC, MAXSEG], F16, tag="x", name=f"x_{t}")[
                        :, :, :lt
                    ]
                    for t, lt in enumerate(SEGS)
                ]
            l0 = 0
            for t, lt in enumerate(SEGS):
                if USE_FP8:
                    nc.sync.dma_start(x_tiles[t], xr[:, :, :, l0 : l0 + lt])
                else:
                    nc.sync.dma_start(x_tiles[t], xr[:, :, l0 : l0 + lt])
                l0 += lt

            carry = [None] * NET  # [128, 1] AP of the previous h column
            pending_store = None  # delayed one unit so stores never block loads

            l0 = 0
            for t, lt in enumerate(SEGS):
                x_sb = x_tiles[t]
                h2 = hpool.tile([128, NET, MAXSEG], F16, tag="h", name=f"h_{t}")
                for et in range(NET):
                    esl = slice(et * 128, (et + 1) * 128)
                    esl_c = slice(EH + et * 128, EH + (et + 1) * 128)
                    a_t = work.tile(
                        [128, MAXSEG], F16, tag=f"a{et}", name=f"a{et}_{t}"
                    )[:, :lt]
                    c_t = work.tile(
                        [128, MAXSEG], F16, tag=f"c{et}", name=f"c{et}_{t}"
                    )[:, :lt]
                    # 1024-token PSUM passes fill the scan unit. Separate
                    # pg/pc tags: a merged 4-bank tile was measured to
                    # serialize the MM stream (+14us on the PE).
                    for p0 in range(0, lt, PSEG):
                        pw = min(PSEG, lt - p0)
                        pg = psum.tile(
                            [128, PSEG], FP32, tag="pg", name=f"pg{et}_{t}_{p0}"
                        )
                        pc = psum.tile(
                            [128, PSEG], FP32, tag="pc", name=f"pc{et}_{t}_{p0}"
                        )
                        for n0 in range(0, pw, NSUB):
                            w = min(NSUB, pw - n0)
                            xsl = slice(p0 + n0, p0 + n0 + w)
                            if USE_FP8:
                                # 3 fp8 terms x 2 dc-pairs per output bank:
                                # xh*Wh + xh*Wl + xl*Wh (lo*lo dropped)
                                for pt, (hx, hw) in enumerate(
                                    [(0, 0), (0, 1), (1, 0)]
                                ):
                                    for dcp in range(NDC // 2):
                                        dsl = slice(2 * dcp, 2 * dcp + 2)
                                        nc.tensor.matmul(
                                            pg[:, n0 : n0 + w],
                                            w_sb[:, hw, dsl, esl],
                                            x_sb[:, hx, dsl, xsl],
                                            start=(pt == 0 and dcp == 0),
                                            stop=(pt == 2 and dcp == 1),
                                            perf_mode=DR,
                                        )
                                for pt, (hx, hw) in enumerate(
                                    [(0, 0), (0, 1), (1, 0)]
                                ):
                                    for dcp in range(NDC // 2):
                                        dsl = slice(2 * dcp, 2 * dcp + 2)
                                        nc.tensor.matmul(
                                            pc[:, n0 : n0 + w],
                                            w_sb[:, hw, dsl, esl_c],
                                            x_sb[:, hx, dsl, xsl],
                                            start=(pt == 0 and dcp == 0),
                                            stop=(pt == 2 and dcp == 1),
                                            perf_mode=DR,
                                        )
                            else:
                                for dc in range(NDC):
                                    nc.tensor.matmul(
                                        pg[:, n0 : n0 + w],
                                        w_sb[:, dc, esl],
                                        x_sb[:, dc, xsl],
                                        start=(dc == 0),
                                        stop=(dc == NDC - 1),
                                    )
                                for dc in range(NDC):
                                    nc.tensor.matmul(
                                        pc[:, n0 : n0 + w],
                                        w_sb[:, dc, esl_c],
                                        x_sb[:, dc, xsl],
                                        start=(dc == 0),
                                        stop=(dc == NDC - 1),
                                    )
                        # a = sigmoid(-(z_g + bg)) = 1 - g ; c = tanh(z_c + bc)
                        nc.scalar.activation(
                            a_t[:, p0 : p0 + pw], pg[:, :pw], act.Sigmoid,
                            bias=bias_sb[:, et : et + 1], scale=-ACT_SCALE,
                        )
                        nc.scalar.activation(
                            c_t[:, p0 : p0 + pw], pc[:, :pw], act.Tanh,
                            bias=bias_sb[:, NET + et : NET + et + 1],
                            scale=ACT_SCALE,
                        )
                    # bn = (a - 1) * c = -g * c in one STT (1x mode), then
                    # h = a * h_prev - bn  (fp32 state in HW, fp16 storage)
                    bn_t = bnpool.tile(
                        [128, MAXSEG], F16, tag=f"b{et}", name=f"b{et}_{t}"
                    )[:, :lt]
                    stt_eng = nc.gpsimd if GPSIMD_STT else nc.vector
                    stt_eng.scalar_tensor_tensor(
                        bn_t, a_t, 1.0, c_t, op.subtract, op.mult
                    )
                    init = 0.0 if carry[et] is None else carry[et]
                    nc.vector.tensor_tensor_scan(
                        h2[:, et, :lt], a_t, bn_t, init, op.mult, op.subtract
                    )
                    carry[et] = h2[:, et, lt - 1 : lt]
                # One store per unit covering both e-tiles, emitted one unit
                # late (sync ring; all x dispatches precede these in program
                # order so stores cannot delay the feed).
                if pending_store is not None:
                    pl0, plt, ph2 = pending_store
                    nc.sync.dma_start(
                        h_pel[:, :, pl0 : pl0 + plt], ph2[:, :, :plt]
                    )
                pending_store = (l0, lt, h2)
                l0 += lt
            pl0, plt, ph2 = pending_store
            nc.sync.dma_start(h_pel[:, :, pl0 : pl0 + plt], ph2[:, :, :plt])
    return nc


def _rearrange_dl(a):
    # [D, N] -> [128, NDC, N]
    n = a.shape[1]
    return np.ascontiguousarray(
        a.reshape(NDC, 128, n).transpose(1, 0, 2)
    )


def _in_maps(x, Wg, bg, Wc, bc):
    F8NP = ml_dtypes.float8_e4m3fn
    maps = []
    xr_cache = {}
    for c in range(NCORES):
        b, eh = c // 2, c % 2
        e0 = eh * EH
        if b not in xr_cache:
            xt = _rearrange_dl(x[b].T)  # [128, NDC, L] fp32
            if USE_FP8:
                xs = xt * np.float32(SX)
                xh = xs.astype(F8NP)
                xl = (xs - xh.astype(np.float32)).astype(F8NP)
                xp = np.empty((128, 2, NDC, L), dtype=F8NP)
                xp[:, 0] = xh
                xp[:, 1] = xl
                xr_cache[b] = xp
            else:
                xr_cache[b] = xt.astype(np.float16)
        bias_pack = np.concatenate(
            [
                (-bg[e0 : e0 + EH]).reshape(NET, 128).T,
                bc[e0 : e0 + EH].reshape(NET, 128).T,
            ],
            axis=1,
        ).astype(np.float32)
        wgc = np.concatenate(
            [Wg[e0 : e0 + EH].T, Wc[e0 : e0 + EH].T], axis=1
        )  # [D, 2*EH]
        wt = _rearrange_dl(wgc)  # [128, NDC, 2*EH]
        if USE_FP8:
            ws = wt * np.float32(SW)
            wh = ws.astype(F8NP)
            wl = (ws - wh.astype(np.float32)).astype(F8NP)
            wp = np.empty((128, 2, NDC, 2 * EH), dtype=F8NP)
            wp[:, 0] = wh
            wp[:, 1] = wl
        else:
            wp = wt.astype(np.float16)
        maps.append(
            {
                "xr": xr_cache[b],
                "w8": wp,
                "bias": np.ascontiguousarray(bias_pack),
            }
        )
    return maps


def kernel(x, Wg, bg, Wc, bc):
    global _last_results
    x = np.asarray(x, dtype=np.float32)
    Wg = np.asarray(Wg, dtype=np.float32)
    bg = np.asarray(bg, dtype=np.float32)
    Wc = np.asarray(Wc, dtype=np.float32)
    bc = np.asarray(bc, dtype=np.float32)

    nc = build_nc()
    if not nc.is_finalized():
        nc.finalize()
    res = run_bass_kernel_spmd(
        nc,
        _in_maps(x, Wg, bg, Wc, bc),
        list(range(NCORES)),
        tmpdir=os.environ.get("KERNEL_TMPDIR"),
    )
    _last_results = res

    out = np.empty((B, L, D), dtype=np.float32)
    for b in range(B):
        hb = np.concatenate(
            [
                res.results[2 * b]["h"].reshape(EH, L),
                res.results[2 * b + 1]["h"].reshape(EH, L),
            ],
            axis=0,
        ).astype(np.float32)
        out[b] = hb.T
    return out
